# revision 49
# baseline (speedup 1.0000x reference)
"""Trainium2 Bass kernel: pre-norm transformer encoder block (B=2,N=2048,D=1024,
Hid=4096,H=16 heads, raw-reshape attention merge, shared LN params).

Sharding (8 cores, no collectives):
  core c: b = c//4, heads h = 4*(c%4)..4*(c%4)+3 of batch b.
  The raw o.reshape(B,N,D) merge maps head h exactly onto tokens
  [128h, 128h+128) of the residual stream, so each core's attention output
  lands on its own contiguous 512-token slice -> MLP is token-parallel with
  zero communication.

Schedule (v3): the kernel is one long software pipeline
  A:  LN1 + qkv (kk, qq-pair0, v) -- PE-bound, ACT mostly idle
  B0: attention pair 0 with (2,1) ping-pong batched exp (PSUM-direct
      [128,2048]+[128,1024] exp reads; ACT-bound) + deferred qq-pair1 fill
  B1: attention pair 1 (plain per-iter exp; PSUM gives 2 banks to the MLP)
      with chunk-0 MLP (LN2+fc1+gelu+fc2 on tokens [0,256)) interleaved into
      the emission stream as filler quanta -> fills PE during the ACT-bound
      exp stream and absorbs the old inter-phase dead zone
  C1: chunk-1 MLP (tokens [256,512)) + stores
Engine discipline: per-engine queues are in-order; any op gated by a DMA
roundtrip goes on gpsimd (never DVE, which feeds exp/PV), epilogue DMA
dispatches go on idle queues per phase. Weight streams (w1 groups, w2
chunks) are single-dispatch batched DMAs prefetched 2 slots deep.
"""

from contextlib import ExitStack

import numpy as np
import ml_dtypes
import bass_rust
import concourse.bass as bass
import concourse.mybir as mybir
import concourse.tile as tile
from concourse.tile import TileContext, ScopedClock
from concourse.bass import ts

F32 = mybir.dt.float32
BF16 = mybir.dt.bfloat16
AF = mybir.ActivationFunctionType
OP = mybir.AluOpType

B, N, D, HID, H = 2, 2048, 1024, 4096, 16
DH = D // H            # 64
NCORES = 8
CPB = 4                # cores per batch
NH = 4                 # heads per core
TOK = N                # tokens per batch (attention span)
MY = 512               # tokens owned per core (MLP/residual)
CH = 256               # MLP chunk tokens (one head pair's residual slice)
P = 128
SL = 512               # free-dim slice for matmuls
NSL = TOK // SL        # 4
KD = D // P            # 8
NKT = TOK // P         # 16
HT = HID // P          # 32
GK = 8                 # w1 column groups
GW = HID // GK         # 512
EPS = 1e-5
EXP_SHIFT = -20.0      # constant logit shift; cancels in softmax, guards overflow

_PATCHED = False


def _patch_drain():
    """This walrus build rejects >2 sem waits on one instruction; split the
    Tile kernel-tail drain's waits across single-wait NOPs."""
    global _PATCHED
    if _PATCHED:
        return
    _PATCHED = True

    def _drain_and_barrier(self, tick_clock, wait_clock):
        gc = tick_clock.global_clock
        ticks = eval(repr(gc).replace("VectorClock", ""))
        n = len(ticks)
        for i, t in enumerate(ticks):
            if t > 0:
                single = [0] * n
                single[i] = t
                vc = bass_rust.VectorClock(single)
                nop = self.nc.sync.nop(nofuse=True, hint=f"drain_split_{i}")
                wait_clock.add_sem_waits(nop.ins, ScopedClock({None: vc}))
        self.nc.sync.drain()
        self.nc.all_engine_barrier()
        assert self.sems is not None
        popped = self.nc._tile_sem_poison_stack.pop()
        assert popped is self._sem_poison
        self.nc.clear_and_free_semaphores(list(self.sems.allocated().values()))
        self.nc.all_engine_barrier()

    TileContext._drain_and_barrier = _drain_and_barrier


def _split_excess_waits(nc):
    """This walrus build supports only one sync wait per instruction (two for
    EventSemaphore). Tile emits more; move the excess onto injected NoOps that
    run just before the instruction on the same engine."""
    nid = [0]
    for fn in nc.m.functions:
        for bb in fn.blocks:
            out = []
            changed = False
            for inst in bb.instructions:
                si = inst.sync_info
                waits = list(si.on_wait) if si is not None and si.on_wait else []
                cap = 2 if inst.opcode == "EventSemaphore" else 1
                if len(waits) > cap:
                    changed = True
                    for w in waits[:-cap]:
                        nid[0] += 1
                        nop = bass_rust.InstNoOp(
                            name=f"I-wsplit{nid[0]}", ins=[], outs=[])
                        nop.engine = inst.engine
                        nop.sync_info = bass_rust.SyncInfo(
                            on_wait=[w], on_update=[])
                        out.append(nop)
                    ups = list(si.on_update) if si.on_update else []
                    inst.sync_info = bass_rust.SyncInfo(
                        on_wait=waits[-cap:], on_update=ups)
                out.append(inst)
            if changed:
                bb.instructions = out


def build_program(split_waits=True):
    _patch_drain()
    nc = bass.Bass()

    xbf = nc.dram_tensor("xbf", [D, TOK], BF16, kind="ExternalInput")
    xTmy = nc.dram_tensor("xTmy", [D, MY], F32, kind="ExternalInput")
    wqk = nc.dram_tensor("wqk", [D, 4 * P], BF16, kind="ExternalInput")
    wv = nc.dram_tensor("wv", [D, NH * DH], BF16, kind="ExternalInput")
    bqk = nc.dram_tensor("bqk", [4 * P], F32, kind="ExternalInput")
    bvx = nc.dram_tensor("bvx", [NH * (DH + 1)], F32, kind="ExternalInput")
    w1 = nc.dram_tensor("w1", [D, HID], BF16, kind="ExternalInput")
    b1 = nc.dram_tensor("b1", [HID], F32, kind="ExternalInput")
    w2 = nc.dram_tensor("w2", [HID, D], BF16, kind="ExternalInput")
    b2 = nc.dram_tensor("b2", [D], F32, kind="ExternalInput")
    outT = nc.dram_tensor("outT", [D, MY], F32, kind="ExternalOutput")

    # DRAM scratch for row->partition broadcasts
    scr_rstd = nc.dram_tensor("scr_rstd", [TOK], BF16)
    scr_negmu = nc.dram_tensor("scr_negmu", [TOK], BF16)
    scr_rcp = nc.dram_tensor("scr_rcp", [16 * SL], F32)
    scr2_rstd = nc.dram_tensor("scr2_rstd", [MY], BF16)
    scr2_negmu = nc.dram_tensor("scr2_negmu", [MY], BF16)

    with TileContext(nc) as tc, ExitStack() as top:
        singles = top.enter_context(tc.tile_pool(name="singles", bufs=1))
        x2T_pool = top.enter_context(tc.tile_pool(name="x2T", bufs=1))

        ones_bf = singles.tile([P, 1], BF16)
        nc.vector.memset(ones_bf, 1.0)
        eps1 = singles.tile([1, 1], F32)
        nc.vector.memset(eps1, EPS)
        shiftP = singles.tile([P, 1], F32)
        nc.vector.memset(shiftP, EXP_SHIFT)
        b1_sb = singles.tile([P, HT], F32)
        nc.sync.dma_start(out=b1_sb, in_=b1.rearrange("(c p) -> p c", p=P))
        b2_sb = singles.tile([P, KD], F32)
        nc.sync.dma_start(out=b2_sb, in_=b2.rearrange("(c p) -> p c", p=P))
        bqk_sb = singles.tile([P, 4], F32)
        nc.sync.dma_start(out=bqk_sb, in_=bqk.rearrange("(c p) -> p c", p=P))
        bvxB = singles.tile([P, NH * (DH + 1)], F32)
        nc.gpsimd.dma_start(
            out=bvxB, in_=bvx[None, :].to_broadcast([P, NH * (DH + 1)]))

        # ================= Phase A: LN1 + qkv + v =================
        esAB = ExitStack()   # pools that live through phase B (qkvT, V')
        qkvT_pool = esAB.enter_context(tc.tile_pool(name="qkvT", bufs=1))
        vsb_pool = esAB.enter_context(tc.tile_pool(name="vsb", bufs=1))

        esA = ExitStack()    # phase-A only
        xn_pool = esA.enter_context(tc.tile_pool(name="xn", bufs=1))
        w_pool = esA.enter_context(tc.tile_pool(name="wp", bufs=1))
        x_pool = esA.enter_context(tc.tile_pool(name="xp", bufs=1))
        xsq_pool = esA.enter_context(tc.tile_pool(name="xsq", bufs=2))
        lnA = esA.enter_context(tc.tile_pool(name="lnA", bufs=1))
        bc_pool = esA.enter_context(tc.tile_pool(name="bcA", bufs=2))

        # resident weights: one batched DMA each ([128, KD, cols] layout)
        wqk_t = w_pool.tile([P, KD, 4 * P], BF16, tag="wqk")
        nc.gpsimd.dma_start(
            out=wqk_t, in_=wqk.rearrange("(kd p) c -> p kd c", p=P))
        wqk_sb = [wqk_t[:, k, :] for k in range(KD)]
        wv_t = w_pool.tile([P, KD, NH * DH], BF16, tag="wv")
        nc.gpsimd.dma_start(
            out=wv_t, in_=wv.rearrange("(kd p) c -> p kd c", p=P))
        wv_sb = [wv_t[:, k, :] for k in range(KD)]

        # full x (bf16), one batched DMA per sl slice; two slots cycle so
        # only ~2 slices are SBUF-resident at a time
        x_sb = [[None] * KD for _ in range(NSL)]
        xr = xbf.rearrange("(kd p) t -> p kd t", p=P)

        def x_load(sl):
            xt = x_pool.tile([P, KD, SL], BF16, name="xt", tag=f"x{sl}")
            nc.sync.dma_start(out=xt, in_=xr[:, :, ts(sl, SL)])
            for k in range(KD):
                x_sb[sl][k] = xt[:, k, :]

        for _sl in range(NSL):
            x_load(_sl)

        # qkvT col-tiles: 0=[q_h0;q_h1] 1=[q_h2;q_h3] 2=[k_h0;k_h1] 3=[k_h2;k_h3]
        qkvT = [qkvT_pool.tile([P, TOK], BF16, name=f"qkvT{ct}", tag=f"qkvT{ct}")
                for ct in range(4)]
        # vsb[nk]: [128 key-tokens, 4*(64+1)]; head h = cols [65h,65h+65),
        # col 65h+64 is the ones column (softmax denominator)
        vsb = [None] * NKT

        xn = [[None] * KD for _ in range(NSL)]

        def ln1_stats(sl, psA, row_pool):
            # squares (bf16 4x DVE)
            xsq = []
            for k in range(KD):
                t = xsq_pool.tile([P, SL], BF16, name="xsq", tag=f"xsq{k}")
                nc.vector.tensor_mul(t, x_sb[sl][k], x_sb[sl][k])
                xsq.append(t)
            s1p = psA.tile([1, SL], F32, tag="s1")
            s2p = psA.tile([1, SL], F32, tag="s2")
            for k in range(KD):
                nc.tensor.matmul(s1p, lhsT=ones_bf, rhs=x_sb[sl][k],
                                 start=(k == 0), stop=(k == KD - 1))
            for k in range(KD):
                nc.tensor.matmul(s2p, lhsT=ones_bf, rhs=xsq[k],
                                 start=(k == 0), stop=(k == KD - 1))
            negmu = row_pool.tile([1, SL], BF16, name="negmu",
                                  tag=f"negmu{sl % 2}")
            nc.vector.tensor_scalar_mul(negmu, s1p, -1.0 / D)
            m2 = row_pool.tile([1, SL], F32, tag="m2")
            nc.vector.tensor_scalar_mul(m2, s2p, 1.0 / D)
            mu2 = row_pool.tile([1, SL], F32, tag="mu2")
            nc.vector.tensor_mul(mu2, negmu, negmu)
            var = row_pool.tile([1, SL], F32, name="var", tag="lnv")
            nc.vector.tensor_sub(var, m2, mu2)
            # rstd = exp(-0.5*ln(var+eps)); Ln and Exp share one ACT table
            lnv = row_pool.tile([1, SL], F32, name="lnv", tag="m2")
            nc.scalar.activation(out=lnv, in_=var, func=AF.Ln,
                                 bias=eps1, scale=1.0)
            rstd = row_pool.tile([1, SL], BF16, name="rstd",
                                 tag=f"rstd{sl % 2}")
            nc.scalar.activation(out=rstd, in_=lnv, func=AF.Exp,
                                 bias=0.0, scale=-0.5)
            nc.scalar.dma_start(out=scr_rstd[ts(sl, SL)], in_=rstd)
            nc.scalar.dma_start(out=scr_negmu[ts(sl, SL)], in_=negmu)
            rstdB = bc_pool.tile([P, SL], BF16, tag="rstdB")
            nc.gpsimd.dma_start(
                out=rstdB,
                in_=scr_rstd[ts(sl, SL)][None, :].to_broadcast([P, SL]))
            negmuB = bc_pool.tile([P, SL], BF16, tag="negmuB")
            nc.gpsimd.dma_start(
                out=negmuB,
                in_=scr_negmu[ts(sl, SL)][None, :].to_broadcast([P, SL]))
            # xn = (x - mu) * rstd   (bf16 4x DVE)
            for k in range(KD):
                xc = xsq_pool.tile([P, SL], BF16, name="xc", tag=f"xc{k}")
                nc.vector.tensor_add(xc, x_sb[sl][k], negmuB)
                t = xn_pool.tile([P, SL], BF16, name="xn", tag=f"xn{sl}_{k}")
                nc.vector.tensor_mul(t, xc, rstdB)
                xn[sl][k] = t

        def qkv_col(sl, ct, psQ):
            pq = psQ.tile([P, SL], F32, tag="pq")
            for k in range(KD):
                nc.tensor.matmul(
                    pq, lhsT=wqk_sb[k][:, ts(ct, P)], rhs=xn[sl][k],
                    start=(k == 0), stop=(k == KD - 1))
            nc.scalar.activation(
                out=qkvT[ct][:, ts(sl, SL)], in_=pq, func=AF.Identity,
                bias=bqk_sb[:, ct:ct + 1], scale=1.0)

        def qkv_v(sl, psQ, psV):
            # kk columns first so attention can start earliest
            for ct in (2, 3, 0, 1):
                qkv_col(sl, ct, psQ)
            for nkl in range(SL // P):
                nk = (SL // P) * sl + nkl
                pv = psV.tile([P, NH * DH], F32, tag="pv")
                for k in range(KD):
                    nc.tensor.matmul(
                        pv, lhsT=xn[sl][k][:, ts(nkl, P)], rhs=wv_sb[k],
                        start=(k == 0), stop=(k == KD - 1))
                vt = vsb_pool.tile([P, NH * (DH + 1)], BF16,
                                   name=f"v{nk}", tag=f"v{nk}")
                vr = vt.rearrange("p (h c) -> p h c", c=DH + 1)
                nc.vector.tensor_add(
                    vr[:, :, 0:DH],
                    pv.rearrange("p (h c) -> p h c", c=DH),
                    bvxB.rearrange("p (h c) -> p h c", c=DH + 1)[:, :, 0:DH])
                nc.vector.memset(vr[:, :, DH:DH + 1], 1.0)
                vsb[nk] = vt

        with (
            tc.tile_pool(name="psA", bufs=2, space="PSUM") as psA,
            tc.tile_pool(name="psQ", bufs=2, space="PSUM") as psQ,
            tc.tile_pool(name="psV", bufs=2, space="PSUM") as psV,
            tc.tile_pool(name="rowA", bufs=1) as rowA,
        ):
            # software-pipelined: stats(sl) ; qkv_v(sl-1)
            ln1_stats(0, psA, rowA)
            for sl in range(1, NSL):
                ln1_stats(sl, psA, rowA)
                qkv_v(sl - 1, psQ, psV)
            qkv_v(NSL - 1, psQ, psV)

        esA.close()   # free x, xsq, xn, weights, LN1 rows

        # ================= Phase B: attention =================
        x2T = [x2T_pool.tile([P, MY], F32, name=f"x2T{k}", tag=f"x2T{k}")
               for k in range(KD)]
        NI = NSL * NKT   # 64 flattened (sl, nk) iterations per pair

        esB = ExitStack()
        oT_pool = esB.enter_context(tc.tile_pool(name="oT", bufs=1))
        rcp_pool = esB.enter_context(tc.tile_pool(name="rcp", bufs=2))
        xTmy_pool = esB.enter_context(tc.tile_pool(name="xTmyp", bufs=1))
        # MLP-persistent pools (created after phase A frees its SBUF)
        w1_pool = esB.enter_context(tc.tile_pool(name="w1sb", bufs=1))
        w2_pool = esB.enter_context(tc.tile_pool(name="w2sb", bufs=1))
        hT_pool = esB.enter_context(tc.tile_pool(name="hT", bufs=1))
        mlp_sb = esB.enter_context(tc.tile_pool(name="mlpsb", bufs=1))
        ln2bc = esB.enter_context(tc.tile_pool(name="ln2bc", bufs=2))
        fctmp_pool = esB.enter_context(tc.tile_pool(name="fctmp", bufs=1))

        w1r = w1.rearrange("(kd p) h -> p kd h", p=P)
        w2r = w2.rearrange("(c p) d -> p c d", p=P)   # [128, 32, 1024]

        def w1_dma(gk):
            # one batched dispatch per group: [128, 8, 512]
            t = w1_pool.tile([P, KD, GW], BF16, name="w1t", tag=f"w1g{gk % 2}")
            nc.gpsimd.dma_start(out=t, in_=w1r[:, :, ts(gk, GW)])
            return [t[:, k, :] for k in range(KD)]

        def w2_dma(c):
            kd, half = divmod(c, 2)
            t = w2_pool.tile([P, HT // 2, P], BF16, name="w2t",
                             tag=f"w2s{c % 4}")
            nc.sync.dma_start(
                out=t, in_=w2r[:, ts(half, HT // 2), ts(kd, P)])
            return t

        xTmy_t = xTmy_pool.tile([P, KD, MY], F32, tag="xTmy")
        nc.sync.dma_start(
            out=xTmy_t, in_=xTmy.rearrange("(kd p) t -> p kd t", p=P))
        xTmy_sb = [xTmy_t[:, k, :] for k in range(KD)]

        # (2,1) ping-pong exp batching: groups of 2 iters ([128,2048] PSUM,
        # 4 banks) alternate with groups of 1 ([128,1024], 2 banks): ACT
        # reads one tile while the PE fills the other, so batched exp never
        # WAR-stalls and the ~352-cycle ACT per-instruction overhead
        # amortizes.  S emission runs one group AHEAD of E so the exp stream
        # never waits on the serial E->PP->S chain.
        sizes = [2, 1] * 21 + [1]
        groups = []
        _i0 = 0
        for _sz in sizes:
            groups.append((_i0, _sz))
            _i0 += _sz
        NGRP = len(groups)
        gof = [None] * NI
        for _gi, (_g0, _sz) in enumerate(groups):
            for _j in range(_sz):
                gof[_g0 + _j] = (_gi, _j)

        def sl_tail(st, sl):
            # drain PSUM promptly, straight into the (unnormalized) output
            # tile; denominator rows go to the per-pair DRAM scratch region
            # (read back [128,32] for one batched reciprocal at pair end)
            off = 2 * NSL * SL * st["pair"]
            for h in range(2):
                oTs = st["oTs2"][h]
                nc.vector.tensor_copy(oTs[0:DH + 1, ts(sl, SL)],
                                      st["po2"][sl][:, ts(h, SL)])
                nc.sync.dma_start(
                    out=scr_rcp[off + (2 * sl + h) * SL:
                                off + (2 * sl + h + 1) * SL],
                    in_=oTs[DH:DH + 1, ts(sl, SL)])

        def make_pair_state(pair):
            return {
                "pair": pair,
                "oTs2": [oT_pool.tile([P, TOK], F32, name=f"oTs{pair}_{h}",
                                      tag=f"oT{pair}{h}") for h in range(2)],
                "po2": [None] * NSL,
            }

        def pp_mm(po2, nk, h0, rhs0, rhs1):
            nc.tensor.matmul(
                po2[:, 0:SL], lhsT=vsb[nk][:, ts(h0, DH + 1)], rhs=rhs0,
                start=(nk == 0), stop=(nk == NKT - 1))
            nc.tensor.matmul(
                po2[:, SL:2 * SL], lhsT=vsb[nk][:, ts(h0 + 1, DH + 1)],
                rhs=rhs1, start=(nk == 0), stop=(nk == NKT - 1))

        esAtt = ExitStack()
        psSb = esAtt.enter_context(
            tc.tile_pool(name="psSb", bufs=1, space="PSUM"))
        psSs = esAtt.enter_context(
            tc.tile_pool(name="psSs", bufs=1, space="PSUM"))
        psO = esAtt.enter_context(
            tc.tile_pool(name="psO", bufs=1, space="PSUM"))
        pTb_pool = esAtt.enter_context(tc.tile_pool(name="pTb", bufs=2))
        pTs_pool = esAtt.enter_context(tc.tile_pool(name="pTs", bufs=2))

        def attention_pair(pair, st, mid_hook=None):
            qq = qkvT[pair]
            kk = qkvT[2 + pair]
            h0 = 2 * pair
            ps_l = [None] * NGRP
            pt_l = [None] * NGRP

            def Sgrp(gi):
                g0, gsz = groups[gi]
                pool = psSb if gsz == 2 else psSs
                ps2 = pool.tile([P, gsz * 2 * SL], F32, name="ps", tag="ps")
                ps_l[gi] = ps2
                for j in range(gsz):
                    sl, nk = divmod(g0 + j, NKT)
                    c0 = j * 2 * SL
                    nc.tensor.matmul(
                        ps2[:, c0:c0 + SL], lhsT=kk[0:64, ts(nk, P)],
                        rhs=qq[0:64, ts(sl, SL)],
                        start=True, stop=True, tile_position=(0, 0))
                    nc.tensor.matmul(
                        ps2[:, c0 + SL:c0 + 2 * SL],
                        lhsT=kk[64:128, ts(nk, P)],
                        rhs=qq[64:128, ts(sl, SL)],
                        start=True, stop=True, tile_position=(64, 0))

            def E(gi):
                gsz = groups[gi][1]
                pool = pTb_pool if gsz == 2 else pTs_pool
                pt = pool.tile([P, gsz * 2 * SL], BF16, name="pt", tag="pt")
                nc.scalar.activation(out=pt, in_=ps_l[gi], func=AF.Exp,
                                     bias=shiftP, scale=1.0)
                ps_l[gi] = None
                pt_l[gi] = pt

            def PPgrp(gi):
                g0, gsz = groups[gi]
                pt = pt_l[gi]
                for j in range(gsz):
                    i = g0 + j
                    sl, nk = divmod(i, NKT)
                    if nk == 0:
                        st["po2"][sl] = psO.tile([DH + 1, 2 * SL], F32,
                                                 name="po2", tag="po2")
                    c0 = j * 2 * SL
                    pp_mm(st["po2"][sl], nk, h0,
                          pt[:, c0:c0 + SL], pt[:, c0 + SL:c0 + 2 * SL])
                    if nk == NKT - 1:
                        sl_tail(st, sl)
                pt_l[gi] = None

            Sgrp(0)
            Sgrp(1)
            for gi in range(NGRP):
                E(gi)
                if gi + 2 < NGRP:
                    Sgrp(gi + 2)
                if gi > 0:
                    PPgrp(gi - 1)
                if gi == 30 and mid_hook is not None:
                    mid_hook()
            PPgrp(NGRP - 1)

        def epilogue(pair, st):
            # one batched reciprocal for all 8 denominators, spread over 128
            # partitions via a DRAM roundtrip (DVE reciprocal cost scales
            # with free size only).  Dispatches on a queue idle in this
            # pair's shadow window; post-DMA elementwise work stays off
            # whatever engine the overlapping phase leans on.
            dq = nc.gpsimd if pair == 0 else nc.scalar
            oTs2 = st["oTs2"]
            off = 2 * NSL * SL * pair
            scr = scr_rcp[off:off + 2 * NSL * SL]
            denp = oT_pool.tile([P, 2 * NSL * SL // P], F32, tag="denp")
            dq.dma_start(out=denp, in_=scr.rearrange("(p c) -> p c", p=P))
            rcpp = oT_pool.tile([P, 2 * NSL * SL // P], F32, tag="rcpp")
            nc.vector.reciprocal(rcpp, denp)
            dq.dma_start(out=scr.rearrange("(p c) -> p c", p=P), in_=rcpp)
            epi = nc.gpsimd if pair == 0 else nc.vector
            for sl in range(NSL):
                for h in range(2):
                    slot = 2 * sl + h
                    rcpB = rcp_pool.tile([DH, SL], F32, tag=f"rcpB{h}")
                    nc.sync.dma_start(
                        out=rcpB,
                        in_=scr[slot * SL:(slot + 1) * SL][None, :]
                        .to_broadcast([DH, SL]))
                    oTs = oTs2[h]
                    epi.tensor_mul(oTs[0:64, ts(sl, SL)],
                                   oTs[0:64, ts(sl, SL)], rcpB)
                    nc.sync.dma_start(out=oTs[64:128, ts(sl, SL)],
                                      in_=oTs[0:64, ts(sl, SL)])
            # scatter both heads' outputs into x2T via strided views:
            # attn_out^T[64j+d, m] = oT[d, 16m+j]
            for h in range(2):
                eng = nc.gpsimd if (pair == 0 or h == 0) else nc.vector
                hh = 2 * pair + h
                c0 = P * hh
                ov = oTs2[h].rearrange("p (m j) -> p m j", j=16)
                for k in range(KD):
                    eng.tensor_add(
                        x2T[k][0:64, c0:c0 + P],
                        xTmy_sb[k][0:64, c0:c0 + P],
                        ov[0:64, :, 2 * k])
                    eng.tensor_add(
                        x2T[k][64:128, c0:c0 + P],
                        xTmy_sb[k][64:128, c0:c0 + P],
                        ov[64:128, :, 2 * k + 1])

        ln2_fut = [None, None]

        def ln2_pre(ch):
            # DVE-only prework (bf16 cast + squares) for the LN2 of chunk
            # ch; safe to emit mid-attention once the chunk's scatter landed
            t0 = CH * ch
            x2h, xsq2 = [], []
            for k in range(KD):
                t = mlp_sb.tile([P, CH], BF16, name="x2h", tag=f"x2h{k}")
                nc.vector.tensor_copy(t, x2T[k][:, t0:t0 + CH])
                x2h.append(t)
            for k in range(KD):
                t = mlp_sb.tile([P, CH], BF16, name="xsq2",
                                tag=f"xq2{k}")
                nc.vector.tensor_mul(t, x2h[k], x2h[k])
                xsq2.append(t)
            ln2_fut[ch] = (x2h, xsq2)

        # ---- pair 0 ----
        st0 = make_pair_state(0)
        attention_pair(0, st0)
        # prefetch first MLP weight groups; epilogue-0 executes during pair 1
        w1q = [w1_dma(0), w1_dma(1)]
        w2_t = [w2_dma(0), w2_dma(1), None, None]
        epilogue(0, st0)

        # ---- pair 1 ----
        st1 = make_pair_state(1)
        attention_pair(1, st1, mid_hook=lambda: ln2_pre(0))
        esAtt.close()   # free attention PSUM before the MLP pools open

        # ================= Phase C: MLP =================
        # fc1 runs per 256-token chunk (chunk 0 is ready as soon as the
        # pair-0 epilogue lands, so it fills the pair-1 epilogue's latency);
        # fc2 runs unified over all 512 tokens (N=512 streams ~25% more
        # efficiently per element than N=256).
        x2b_c = [[None] * KD for _ in range(2)]
        hT = [None] * HT

        with tc.tile_pool(name="psF", bufs=2, space="PSUM") as psF:

            def ln2_chain(ch):
                t0 = CH * ch
                if ln2_fut[ch] is None:
                    ln2_pre(ch)
                x2h, xsq2 = ln2_fut[ch]
                st = psF.tile([P, 2 * CH], F32, tag="pf")
                for k in range(KD):
                    nc.tensor.matmul(st[0:1, 0:CH], lhsT=ones_bf, rhs=x2h[k],
                                     start=(k == 0), stop=(k == KD - 1))
                for k in range(KD):
                    nc.tensor.matmul(st[0:1, CH:2 * CH], lhsT=ones_bf,
                                     rhs=xsq2[k],
                                     start=(k == 0), stop=(k == KD - 1))
                negmu2 = mlp_sb.tile([1, CH], BF16, tag="negmu2")
                nc.vector.tensor_scalar_mul(negmu2, st[0:1, 0:CH], -1.0 / D)
                m2 = mlp_sb.tile([1, CH], F32, tag="m2b")
                nc.vector.tensor_scalar_mul(m2, st[0:1, CH:2 * CH], 1.0 / D)
                mu22 = mlp_sb.tile([1, CH], F32, tag="mu22")
                nc.vector.tensor_mul(mu22, negmu2, negmu2)
                var = mlp_sb.tile([1, CH], F32, name="var2", tag="lnv2")
                nc.vector.tensor_sub(var, m2, mu22)
                lnv2 = mlp_sb.tile([1, CH], F32, name="lnv2", tag="m2b")
                nc.scalar.activation(out=lnv2, in_=var, func=AF.Ln,
                                     bias=eps1, scale=1.0)
                rstd2 = mlp_sb.tile([1, CH], BF16, tag="rstd2")
                nc.scalar.activation(out=rstd2, in_=lnv2, func=AF.Exp,
                                     bias=0.0, scale=-0.5)
                nc.gpsimd.dma_start(out=scr2_rstd[t0:t0 + CH], in_=rstd2)
                nc.gpsimd.dma_start(out=scr2_negmu[t0:t0 + CH], in_=negmu2)
                rstd2B = ln2bc.tile([P, CH], BF16, tag="rstd2B")
                nc.gpsimd.dma_start(
                    out=rstd2B,
                    in_=scr2_rstd[t0:t0 + CH][None, :].to_broadcast([P, CH]))
                negmu2B = ln2bc.tile([P, CH], BF16, tag="negmu2B")
                nc.gpsimd.dma_start(
                    out=negmu2B,
                    in_=scr2_negmu[t0:t0 + CH][None, :].to_broadcast([P, CH]))
                for k in range(KD):
                    xc = mlp_sb.tile([P, CH], BF16, name="xc2",
                                     tag=f"xq2{k}")
                    nc.vector.tensor_add(xc, x2h[k], negmu2B)
                    t = mlp_sb.tile([P, CH], BF16, name="x2b",
                                    tag=f"x2b{ch}_{k}")
                    nc.vector.tensor_mul(t, xc, rstd2B)
                    x2b_c[ch][k] = t

            def fc1_group(ch, gk, w1sb):
                for half in range(2):
                    pf = psF.tile([P, 2, CH], F32, tag="pf")
                    for j in range(2):
                        khl = 2 * half + j
                        for k in range(KD):
                            nc.tensor.matmul(
                                pf[:, j, :], lhsT=w1sb[k][:, ts(khl, P)],
                                rhs=x2b_c[ch][k],
                                start=(k == 0), stop=(k == KD - 1))
                    for j in range(2):
                        kh = 4 * gk + 2 * half + j
                        if ch == 0:
                            hT[kh] = hT_pool.tile([P, MY], BF16, name="ht",
                                                  tag=f"hT{kh}")
                        nc.scalar.activation(
                            out=hT[kh][:, ts(ch, CH)], in_=pf[:, j, :],
                            func=AF.Gelu, bias=b1_sb[:, kh:kh + 1], scale=1.0)

            def fc2_unified(kd):
                for c in (2 * kd + 2, 2 * kd + 3):
                    if c < 2 * KD:
                        w2_t[c % 4] = w2_dma(c)
                pf = psF.tile([P, MY], F32, tag="pf2")
                for kh in range(HT):
                    half, khl = divmod(kh, HT // 2)
                    w2h = w2_t[(2 * kd + half) % 4]
                    nc.tensor.matmul(pf, lhsT=w2h[:, khl, :], rhs=hT[kh],
                                     start=(kh == 0), stop=(kh == HT - 1))
                t = fctmp_pool.tile([P, MY], F32, tag="fco")
                nc.vector.tensor_scalar_add(t, pf, b2_sb[:, kd:kd + 1])
                ot = fctmp_pool.tile([P, MY], F32, tag="fcout")
                nc.vector.tensor_add(ot, t, x2T[kd])
                nc.gpsimd.dma_start(out=outT[ts(kd, P), :], in_=ot)

            keep = [None, None]

            def fc1_c0(gk):
                w1sb = w1q.pop(0)
                if gk + 2 < GK:
                    w1q.append(w1_dma(gk + 2))
                fc1_group(0, gk, w1sb)
                keep[gk % 2] = w1sb

            # chunk-0 fc1 starts immediately (x2 tokens [0,256) landed with
            # the pair-0 epilogue); the pair-1 epilogue + LN2-c1 roundtrips
            # hide under it
            ln2_chain(0)
            fc1_c0(0)
            epilogue(1, st1)
            fc1_c0(1)
            fc1_c0(2)
            fc1_c0(3)
            ln2_chain(1)
            for gk in range(4, GK):
                fc1_c0(gk)
            # chunk-1 fc1: groups 6,7 reuse the still-resident tiles, then
            # stream 0..5 back in
            fc1_group(1, 6, keep[0])
            w1q = [w1_dma(0), w1_dma(1)]
            fc1_group(1, 7, keep[1])
            for gk in range(6):
                w1sb = w1q.pop(0)
                if gk + 2 < 6:
                    w1q.append(w1_dma(gk + 2))
                fc1_group(1, gk, w1sb)
            for kd in range(KD):
                fc2_unified(kd)
        esB.close()
        esAB.close()  # free qkvT, V'

    if split_waits:
        _split_excess_waits(nc)
    return nc


def host_prep(x, w_qkv, b_qkv, ln_g, ln_b, w1, b1, w2, b2):
    """Fold LN affine params into weights; build per-core input maps."""
    x = np.asarray(x, np.float32)
    w_qkv = np.asarray(w_qkv, np.float32)
    b_qkv = np.asarray(b_qkv, np.float32)
    ln_g = np.asarray(ln_g, np.float32)
    ln_b = np.asarray(ln_b, np.float32)
    w1 = np.asarray(w1, np.float32)
    b1 = np.asarray(b1, np.float32)
    w2 = np.asarray(w2, np.float32)
    b2 = np.asarray(b2, np.float32)

    wqkv_eff = ln_g[:, None] * w_qkv
    bqkv_eff = b_qkv + ln_b @ w_qkv
    w1_eff = np.ascontiguousarray(ln_g[:, None] * w1)
    b1_eff = b1 + ln_b @ w1

    in_maps = []
    for c in range(NCORES):
        b = c // CPB
        heads = [4 * (c % CPB) + i for i in range(NH)]
        qcols = np.concatenate([np.arange(h * DH, (h + 1) * DH) for h in heads])
        kcols = qcols + D
        vcols = qcols + 2 * D
        qkcols = np.concatenate([qcols, kcols])
        xb = x[b]
        my0 = MY * (c % CPB)
        bvx = np.zeros(NH * (DH + 1), np.float32)
        bv = bqkv_eff[vcols]
        for h in range(NH):
            bvx[h * (DH + 1):h * (DH + 1) + DH] = bv[h * DH:(h + 1) * DH]
        in_maps.append({
            "xbf": np.ascontiguousarray(xb.T).astype(ml_dtypes.bfloat16),
            "xTmy": np.ascontiguousarray(xb[my0:my0 + MY].T),
            "wqk": np.ascontiguousarray(
                wqkv_eff[:, qkcols]).astype(ml_dtypes.bfloat16),
            "wv": np.ascontiguousarray(
                wqkv_eff[:, vcols]).astype(ml_dtypes.bfloat16),
            "bqk": np.ascontiguousarray(bqkv_eff[qkcols]),
            "bvx": bvx,
            "w1": w1_eff.astype(ml_dtypes.bfloat16),
            "b1": b1_eff,
            "w2": w2.astype(ml_dtypes.bfloat16),
            "b2": b2,
        })
    return in_maps


_NC_CACHE = None


def kernel(x, w_qkv, b_qkv, ln_g, ln_b, w1, b1, w2, b2):
    global _NC_CACHE
    from concourse.bass_utils import run_bass_kernel_spmd

    if _NC_CACHE is None:
        _NC_CACHE = build_program()
    nc = _NC_CACHE
    in_maps = host_prep(x, w_qkv, b_qkv, ln_g, ln_b, w1, b1, w2, b2)
    res = run_bass_kernel_spmd(nc, in_maps, list(range(NCORES))).results

    out = np.empty((B, N, D), np.float32)
    for c in range(NCORES):
        b = c // CPB
        my0 = MY * (c % CPB)
        out[b, my0:my0 + MY, :] = res[c]["outT"].T
    return out
